# revision 15
# baseline (speedup 1.0000x reference)
"""Trainium2 Bass kernel for nn_DSAG_58025008169585.

Data-parallel over the ego batch dim across 8 NeuronCores: each core gets
EBL=4 ego samples and their BXL=12 exo views (reordered so local exo row
r pairs with local ego row r % 4). Weights replicated. Big GEMMs run in
bf16 with fp32 PSUM accumulation; attention rows, softmax/Sinkhorn, and
classifier heads are fused on-chip in fp32.

Math identities used (vs reference.py):
 - exo branch never uses the attention map -> skip scores/softmax for exo.
 - _l2(ls*v) == _l2(v) and the per-row text norm cancels inside _l2 over s,
   so the att path needs only raw gathered text features.
 - exo_branch is only consumed through mean over s -> never materialized:
   z[o,b] = sum_s scale_b[s]*Y[o,s] with scale folded by 0.5/196, and the
   bias term collapses to c0 = fc_exo_w @ b_aff_exo + fc_exo_b.
 - img mean over s is a free accum_out of the PSUM->SBUF tok copy; its
   1/196 factor cancels in l2 normalization (folded into logits scaling).

Engine APs must start at partition 0, so all per-sample row math happens
in [1,S] base-0 workspaces; batched row tiles are assembled via DMA.
"""

import numpy as np
import ml_dtypes

import concourse.bass as bass
import concourse.tile as tile
from concourse import bacc, mybir
from concourse.bass_utils import run_bass_kernel_spmd
from concourse.masks import make_identity

F32 = mybir.dt.float32
BF16 = mybir.dt.bfloat16
AF = mybir.ActivationFunctionType
ALU = mybir.AluOpType
AX = mybir.AxisListType

EB, NV, C, H, W = 32, 3, 2048, 14, 14
S = H * W                      # 196
E, NCL, O = 1024, 36, 512
NCORES = 8
EBL = EB // NCORES             # 4 ego per core
BXL = EBL * NV                 # 12 exo per core
KC = C // 128                  # 16 contraction chunks
ECH = E // 128                 # 8 tok chunks
OCH = O // 128                 # 4 aff chunks
MC = ECH + OCH                 # 12 weight-column chunks (pool | affT)
S0, S1 = 128, S - 128          # mu partition chunks


def build_nc():
    nc = bacc.Bacc(None, target_bir_lowering=False)

    # ---- DRAM I/O (per-core shapes) ----
    d_xexo = nc.dram_tensor("xexo", [BXL, C, S], BF16, kind="ExternalInput")
    d_xego = nc.dram_tensor("xego", [EBL, C, S], BF16, kind="ExternalInput")
    d_wexo = nc.dram_tensor("wexo", [C, 1536], BF16, kind="ExternalInput")
    d_wego = nc.dram_tensor("wego", [C, 1536], BF16, kind="ExternalInput")
    d_txt = nc.dram_tensor("txt", [NCL, E], F32, kind="ExternalInput")
    d_txtT = nc.dram_tensor("txtT", [E, NCL], F32, kind="ExternalInput")
    d_tgT = nc.dram_tensor("tgT", [E, EBL], BF16, kind="ExternalInput")
    d_fcwgT = nc.dram_tensor("fcwgT", [O, EBL], F32, kind="ExternalInput")
    d_fcwT = nc.dram_tensor("fcwT", [O, NCL], F32, kind="ExternalInput")
    d_fcxT = nc.dram_tensor("fcxT", [O, NCL], F32, kind="ExternalInput")
    d_fcb = nc.dram_tensor("fcb", [1, NCL], F32, kind="ExternalInput")
    d_fcxb = nc.dram_tensor("fcxb", [1, NCL], F32, kind="ExternalInput")
    d_baffx = nc.dram_tensor("baffx", [128, OCH], F32, kind="ExternalInput")
    d_baffe = nc.dram_tensor("baffe", [128, OCH], F32, kind="ExternalInput")
    d_ls = nc.dram_tensor("ls", [1, 2], F32, kind="ExternalInput")

    o_xscore = nc.dram_tensor("xscore", [BXL, NCL], F32, kind="ExternalOutput")
    o_escore = nc.dram_tensor("escore", [EBL, NCL], F32, kind="ExternalOutput")
    o_lpi = nc.dram_tensor("lpi", [EBL, NCL], F32, kind="ExternalOutput")
    o_lpix = nc.dram_tensor("lpix", [BXL, NCL], F32, kind="ExternalOutput")
    o_sim = nc.dram_tensor("sim", [BXL, 1], F32, kind="ExternalOutput")
    o_cam = nc.dram_tensor("cam", [EBL, S], F32, kind="ExternalOutput")
    o_ebr = nc.dram_tensor("ebr", [EBL, O, S], F32, kind="ExternalOutput")
    o_mu = nc.dram_tensor("mu", [EBL, S, S], F32, kind="ExternalOutput")

    with tile.TileContext(nc) as tc:
        with (
            tc.tile_pool(name="konst", bufs=1) as konst,
            tc.tile_pool(name="persist", bufs=1) as persist,
            tc.tile_pool(name="wpool", bufs=1) as wpool,
            tc.tile_pool(name="xpool", bufs=3) as xpool,
            tc.tile_pool(name="tokpool", bufs=3) as tokpool,
            tc.tile_pool(name="sqpool", bufs=3) as sqpool,
            tc.tile_pool(name="bcpool", bufs=3) as bcpool,
            tc.tile_pool(name="rbpool", bufs=2) as rbpool,
            tc.tile_pool(name="scrpool", bufs=2) as scrpool,
            tc.tile_pool(name="rowpool", bufs=4) as rowpool,
            tc.tile_pool(name="mupool", bufs=2) as mupool,
            tc.tile_pool(name="ps_gemm", bufs=5, space="PSUM") as ps_gemm,
            tc.tile_pool(name="ps_sm", bufs=3, space="PSUM") as ps_sm,
        ):
            # ---- constants & small loads ----
            ident = konst.tile([128, 128], F32)
            make_identity(nc, ident)
            ones1b = konst.tile([128, 1], BF16)
            nc.vector.memset(ones1b, 1.0)
            ones1f = konst.tile([128, 1], F32)
            nc.vector.memset(ones1f, 1.0)

            txt_sb = konst.tile([NCL, E], F32)
            nc.sync.dma_start(out=txt_sb[:], in_=d_txt[:])
            txtT_sb = konst.tile([128, ECH, NCL], F32)
            nc.sync.dma_start(out=txtT_sb[:], in_=d_txtT.rearrange("(k p) n -> p k n", p=128))
            tgT_sb = konst.tile([128, ECH, EBL], BF16)
            nc.sync.dma_start(out=tgT_sb[:], in_=d_tgT.rearrange("(k p) n -> p k n", p=128))
            fcwgT_sb = konst.tile([128, OCH, EBL], F32)
            nc.sync.dma_start(out=fcwgT_sb[:], in_=d_fcwgT.rearrange("(k p) n -> p k n", p=128))
            fcwT_sb = konst.tile([128, OCH, NCL], F32)
            nc.sync.dma_start(out=fcwT_sb[:], in_=d_fcwT.rearrange("(k p) n -> p k n", p=128))
            fcxT_sb = konst.tile([128, OCH, NCL], F32)
            nc.sync.dma_start(out=fcxT_sb[:], in_=d_fcxT.rearrange("(k p) n -> p k n", p=128))
            fcb_sb = konst.tile([1, NCL], F32)
            nc.sync.dma_start(out=fcb_sb[:], in_=d_fcb[:])
            fcxb_sb = konst.tile([1, NCL], F32)
            nc.sync.dma_start(out=fcxb_sb[:], in_=d_fcxb[:])
            baffx_sb = konst.tile([128, OCH], F32)
            nc.sync.dma_start(out=baffx_sb[:], in_=d_baffx[:])
            baffe_sb = konst.tile([128, OCH], F32)
            nc.sync.dma_start(out=baffe_sb[:], in_=d_baffe[:])
            lsr_sb = konst.tile([1, 2], F32)
            nc.sync.dma_start(out=lsr_sb[:], in_=d_ls[:])

            ls_e = konst.tile([1, 2], F32)
            nc.scalar.activation(out=ls_e, in_=lsr_sb, func=AF.Exp)
            ls12 = konst.tile([12, 1], F32)
            nc.gpsimd.partition_broadcast(ls12, ls_e[:, 1:2], channels=12)
            ls4 = konst.tile([4, 1], F32)
            nc.gpsimd.partition_broadcast(ls4, ls_e[:, 0:1], channels=4)

            # text norms (for logits column scaling)
            scr_txt = konst.tile([NCL, E], F32)
            ss_txt = konst.tile([NCL, 1], F32)
            nc.vector.tensor_mul(scr_txt, txt_sb, txt_sb)
            nc.vector.reduce_sum(out=ss_txt, in_=scr_txt, axis=AX.X)
            nc.scalar.activation(out=ss_txt, in_=ss_txt, func=AF.Sqrt)
            nc.vector.tensor_scalar_max(ss_txt, ss_txt, 1e-12)
            nc.vector.reciprocal(out=ss_txt, in_=ss_txt)
            itx_ps = ps_sm.tile([1, NCL], F32, tag="sm", name="itx_ps")
            nc.tensor.transpose(itx_ps[:], ss_txt[:], ident[:NCL, :NCL])
            itx_row = konst.tile([1, NCL], F32)
            nc.vector.tensor_copy(out=itx_row, in_=itx_ps)
            itx_bc = konst.tile([12, NCL], F32)
            nc.gpsimd.partition_broadcast(itx_bc, itx_row, channels=12)

            # c0 = fc_exo_w @ b_aff_exo + fc_exo_b  (row [1,36])
            c0_ps = ps_sm.tile([1, NCL], F32, tag="sm", name="c0_ps")
            for k in range(OCH):
                nc.tensor.matmul(
                    c0_ps[:], baffx_sb[:, k:k + 1], fcxT_sb[:, k, :],
                    start=(k == 0), stop=(k == OCH - 1),
                )
            c0_row = konst.tile([1, NCL], F32)
            nc.vector.tensor_add(c0_row, c0_ps, fcxb_sb)
            c0_bc = konst.tile([12, NCL], F32)
            nc.gpsimd.partition_broadcast(c0_bc, c0_row, channels=12)

            # ---- persistent accumulators ----
            img_exo = persist.tile([128, ECH, BXL], F32)
            img_ego = persist.tile([128, ECH, EBL], F32)
            z_exo = persist.tile([128, OCH, BXL], F32)
            z_ego = persist.tile([128, OCH, EBL], F32)
            attexo = persist.tile([12, S], F32)
            attego = persist.tile([4, S], F32)
            ebr = persist.tile([128, OCH, EBL, S], F32)
            m0_t = persist.tile([128, OCH, EBL], F32)
            col_a = persist.tile([128, 1], F32, name="col_a")
            col_b = persist.tile([128, 1], F32, name="col_b")
            bias_xs = persist.tile([1, 1], F32, name="bias_xs")
            nc.vector.memset(bias_xs, 0.5 / 196.0)
            bias_es = persist.tile([1, 1], F32, name="bias_es")
            nc.vector.memset(bias_es, 0.5)

            def pool_gemm(w_sb, xp, tk, img, pair, tagn):
                """M=0..ECH-1 GEMM chunks for one b-pair, drained to bf16 tok
                with the (unnormalized) image mean as a free accum."""
                for M in range(ECH):
                    pt = ps_gemm.tile([128, 392], F32, tag="pt", name=f"pt_{tagn}_{M}")
                    for K in range(KC):
                        nc.tensor.matmul(
                            pt[:], w_sb[:, K, M * 128:(M + 1) * 128], xp[:, K, :, :],
                            start=(K == 0), stop=(K == KC - 1),
                        )
                    for i in range(2):
                        nc.scalar.activation(
                            out=tk[:, M, i, :], in_=pt[:, i * 196:(i + 1) * 196],
                            func=AF.Identity,
                            accum_out=img[:, M, pair * 2 + i:pair * 2 + i + 1],
                        )

            def att_row(tk, i, jcol, att_all, b, bias_sc, sc_scale, sc_bc_half, tagn):
                """One sample: d/ss matvecs -> att row -> scale row -> bcast.

                att = sigmoid(l2_s(d * rsqrt(ss))); scale = att*sc_scale+bias.
                Writes att row into att_all[b] (via DMA) and the broadcast
                scale into sc_bc_half [128,196].
                """
                dps = ps_sm.tile([1, S], F32, tag="sm", name=f"dps_{tagn}")
                for K in range(ECH):
                    nc.tensor.matmul(
                        dps[:], tgT_sb[:, K, jcol:jcol + 1],
                        tk[:, K, i, :], start=(K == 0), stop=(K == ECH - 1),
                    )
                sps = ps_sm.tile([1, S], F32, tag="sm", name=f"sps_{tagn}")
                for K in range(ECH):
                    sq = sqpool.tile([128, 196], BF16, tag="sq", name=f"sq_{tagn}_{K}")
                    nc.vector.tensor_mul(sq[:], tk[:, K, i, :], tk[:, K, i, :])
                    nc.tensor.matmul(
                        sps[:], ones1b[:], sq[:],
                        start=(K == 0), stop=(K == ECH - 1),
                    )
                ns = rowpool.tile([1, S], F32, tag="ns", name=f"ns_{tagn}")
                ap = rowpool.tile([1, S], F32, tag="ap", name=f"ap_{tagn}")
                scr = rowpool.tile([1, S], F32, tag="rscr", name=f"rscr_{tagn}")
                s2 = rowpool.tile([1, 1], F32, tag="s2", name=f"s2_{tagn}")
                nc.scalar.activation(out=ns, in_=sps, func=AF.Sqrt)
                nc.vector.tensor_scalar_max(ns, ns, 1e-12)
                nc.vector.reciprocal(out=ns, in_=ns)
                nc.vector.tensor_mul(ap, dps, ns)
                nc.vector.tensor_mul(scr, ap, ap)
                nc.vector.reduce_sum(out=s2, in_=scr, axis=AX.X)
                nc.scalar.activation(out=s2, in_=s2, func=AF.Sqrt)
                nc.vector.tensor_scalar_max(s2, s2, 1e-12)
                nc.vector.reciprocal(out=s2, in_=s2)
                nc.vector.tensor_scalar_mul(ap, ap, s2)
                att_b = rowpool.tile([1, S], F32, tag="att", name=f"att_{tagn}")
                nc.scalar.activation(out=att_b, in_=ap, func=AF.Sigmoid)
                nc.sync.dma_start(out=att_all[b:b + 1, :], in_=att_b[:])
                scale_b = rowpool.tile([1, S], F32, tag="scl", name=f"scl_{tagn}")
                nc.scalar.activation(out=scale_b, in_=att_b, func=AF.Identity,
                                     scale=sc_scale, bias=bias_sc)
                nc.gpsimd.partition_broadcast(sc_bc_half, scale_b, channels=128)

            # ================= EXO phase =================
            w_exo = wpool.tile([128, KC, 1536], BF16, tag="w", name="w_exo")
            for K in range(KC):
                nc.sync.dma_start(out=w_exo[:, K, :], in_=d_wexo[K * 128:(K + 1) * 128, :])

            for pair in range(6):
                b0 = 2 * pair
                xp = xpool.tile([128, KC, 2, 196], BF16, tag="x", name=f"xp_x{pair}")
                for i in range(2):
                    nc.sync.dma_start(
                        out=xp[:, :, i, :],
                        in_=d_xexo[b0 + i].rearrange("(k p) s -> p k s", p=128),
                    )
                tk = tokpool.tile([128, ECH, 2, 196], BF16, tag="tok", name=f"tk_x{pair}")
                pool_gemm(w_exo, xp, tk, img_exo, pair, f"x{pair}")

                sc_bc = bcpool.tile([128, 2, 196], F32, tag="bc", name=f"scbc_x{pair}")
                for i in range(2):
                    att_row(tk, i, (b0 + i) % 4, attexo, b0 + i, bias_xs,
                            0.5 / 196.0, sc_bc[:, i, :], f"x{pair}_{i}")

                for Mc in range(OCH):
                    M = ECH + Mc
                    pt = ps_gemm.tile([128, 392], F32, tag="pt", name=f"pt_x{pair}_{M}")
                    for K in range(KC):
                        nc.tensor.matmul(
                            pt[:], w_exo[:, K, M * 128:(M + 1) * 128], xp[:, K, :, :],
                            start=(K == 0), stop=(K == KC - 1),
                        )
                    for i in range(2):
                        scr = scrpool.tile([128, 196], F32, tag="scr", name=f"scr_x{pair}_{Mc}_{i}")
                        nc.vector.tensor_mul(scr[:], pt[:, i * 196:(i + 1) * 196], sc_bc[:, i, :])
                        nc.vector.reduce_sum(out=z_exo[:, Mc, b0 + i:b0 + i + 1],
                                             in_=scr[:], axis=AX.X)

            # ---- exo epilogue: exo_score + logits_exo ----
            xs_ps = ps_sm.tile([12, NCL], F32, tag="sm", name="xs_ps")
            for Kc in range(OCH):
                nc.tensor.matmul(xs_ps[:], z_exo[:, Kc, :], fcxT_sb[:, Kc, :],
                                 start=(Kc == 0), stop=(Kc == OCH - 1))
            xs_sb = persist.tile([12, NCL], F32)
            nc.vector.tensor_add(xs_sb, xs_ps, c0_bc)
            nc.sync.dma_start(out=o_xscore[:], in_=xs_sb[:])

            def logits(img, nb, ls_col, out_dram, name):
                lp_ps = ps_sm.tile([nb, NCL], F32, tag="sm", name=f"lp_{name}")
                for K in range(ECH):
                    nc.tensor.matmul(lp_ps[:], img[:, K, :], txtT_sb[:, K, :],
                                     start=(K == 0), stop=(K == ECH - 1))
                si_ps = ps_sm.tile([1, nb], F32, tag="sm", name=f"si_{name}")
                for K in range(ECH):
                    isq = scrpool.tile([128, nb], F32, tag="scr", name=f"isq_{name}_{K}")
                    nc.vector.tensor_mul(isq[:], img[:, K, :], img[:, K, :])
                    nc.tensor.matmul(si_ps[:], ones1f[:], isq[:],
                                     start=(K == 0), stop=(K == ECH - 1))
                # [1,nb] -> [nb,1] via a 32-row padded PE transpose (K=1 is a
                # HW crasher; K=32 is safe)
                si_pad = persist.tile([32, nb], F32, name=f"sipad_{name}")
                nc.vector.memset(si_pad, 0.0)
                nc.scalar.activation(out=si_pad[0:1, :], in_=si_ps, func=AF.Sqrt)
                nc.vector.tensor_scalar_max(si_pad[0:1, :], si_pad[0:1, :], 1e-12)
                nc.vector.reciprocal(out=si_pad[0:1, :], in_=si_pad[0:1, :])
                ii_ps = ps_sm.tile([nb, 32], F32, tag="sm", name=f"ii_{name}")
                nc.tensor.transpose(ii_ps[:], si_pad[:], ident[:32, :32])
                ii_col = persist.tile([nb, 1], F32, name=f"iicol_{name}")
                nc.vector.tensor_copy(out=ii_col, in_=ii_ps[:, 0:1])
                lg = persist.tile([nb, NCL], F32, name=f"lg_{name}")
                nc.vector.tensor_scalar(
                    out=lg, in0=lp_ps, scalar1=ii_col, scalar2=ls_col,
                    op0=ALU.mult, op1=ALU.mult,
                )
                nc.vector.tensor_mul(lg, lg, itx_bc[:nb, :])
                nc.sync.dma_start(out=out_dram[:], in_=lg[:])

            logits(img_exo, BXL, ls12, o_lpix, "exo")

            # ================= EGO phase =================
            w_ego = wpool.tile([128, KC, 1536], BF16, tag="w", name="w_ego")
            for K in range(KC):
                nc.sync.dma_start(out=w_ego[:, K, :], in_=d_wego[K * 128:(K + 1) * 128, :])

            for pair in range(2):
                b0 = 2 * pair
                xp = xpool.tile([128, KC, 2, 196], BF16, tag="x", name=f"xp_e{pair}")
                for i in range(2):
                    nc.sync.dma_start(
                        out=xp[:, :, i, :],
                        in_=d_xego[b0 + i].rearrange("(k p) s -> p k s", p=128),
                    )
                tk = tokpool.tile([128, ECH, 2, 196], BF16, tag="tok", name=f"tk_e{pair}")
                pool_gemm(w_ego, xp, tk, img_ego, pair, f"e{pair}")

                sc_bc = bcpool.tile([128, 2, 196], F32, tag="bc", name=f"scbc_e{pair}")
                for i in range(2):
                    att_row(tk, i, b0 + i, attego, b0 + i, bias_es,
                            0.5, sc_bc[:, i, :], f"e{pair}_{i}")

                # scores -> softmax -> sinkhorn -> sym -> mu@mu, per b
                for i in range(2):
                    b = b0 + i
                    mu_t = mupool.tile([128, 2, S], F32, tag="mu", name=f"mu_{b}")
                    for sc, (p0, pw) in enumerate(((0, S0), (S0, S1))):
                        sps = ps_sm.tile([128, S], F32, tag="sm", name=f"sc_{b}_{sc}")
                        for K in range(ECH):
                            nc.tensor.matmul(
                                sps[:pw, :], tk[:, K, i, p0:p0 + pw], tk[:, K, i, :],
                                start=(K == 0), stop=(K == ECH - 1),
                            )
                        nm = col_a[:pw, :]
                        nc.vector.tensor_reduce(out=nm, in_=sps[:pw, :], axis=AX.X,
                                                op=ALU.max)
                        nc.vector.tensor_scalar_mul(nm, nm, -1.0 / 32.0)
                        rs = col_b[:pw, :]
                        nc.scalar.activation(out=mu_t[:pw, sc, :], in_=sps[:pw, :],
                                             func=AF.Exp, scale=1.0 / 32.0, bias=nm,
                                             accum_out=rs)
                        nc.vector.reciprocal(out=rs, in_=rs)
                        nc.vector.tensor_scalar_mul(mu_t[:pw, sc, :], mu_t[:pw, sc, :], rs)

                    # Sinkhorn: 3x (normalize over s [partitions], then t [free])
                    for it in range(3):
                        cs_ps = ps_sm.tile([1, S], F32, tag="sm", name=f"cs_{b}_{it}")
                        nc.tensor.matmul(cs_ps[:], ones1f[:], mu_t[:, 0, :], start=True, stop=False)
                        nc.tensor.matmul(cs_ps[:], ones1f[:S1], mu_t[:S1, 1, :], start=False, stop=True)
                        cr = rowpool.tile([1, S], F32, tag="rscr", name=f"cr_{b}_{it}")
                        nc.vector.reciprocal(out=cr, in_=cs_ps)
                        rb = rbpool.tile([128, S], F32, tag="rb", name=f"rb_{b}_{it}")
                        nc.gpsimd.partition_broadcast(rb, cr, channels=128)
                        nc.vector.tensor_mul(mu_t[:, 0, :], mu_t[:, 0, :], rb[:, :])
                        nc.vector.tensor_mul(mu_t[:S1, 1, :], mu_t[:S1, 1, :], rb[:S1, :])
                        for sc, pw in ((0, S0), (1, S1)):
                            rsum = col_b[:pw, :]
                            nc.vector.tensor_reduce(out=rsum, in_=mu_t[:pw, sc, :],
                                                    axis=AX.X, op=ALU.add)
                            nc.vector.reciprocal(out=rsum, in_=rsum)
                            nc.vector.tensor_scalar_mul(mu_t[:pw, sc, :], mu_t[:pw, sc, :], rsum)

                    # symmetrize: P = mu + mu^T  (output mu = 0.25 * P @ P)
                    muT = mupool.tile([128, 2, S], F32, tag="muT", name=f"muT_{b}")
                    blocks = [
                        (mu_t[:, 0, :S0], muT[:, 0, :S0], S0, S0),
                        (mu_t[:, 0, S0:], muT[:S1, 1, :S0], S0, S1),
                        (mu_t[:S1, 1, :S0], muT[:, 0, S0:], S1, S0),
                        (mu_t[:S1, 1, S0:], muT[:S1, 1, S0:], S1, S1),
                    ]
                    for bi, (src, dst, pin, pout) in enumerate(blocks):
                        tp = ps_sm.tile([128, 128], F32, tag="sm", name=f"tp_{b}_{bi}")
                        nc.tensor.transpose(tp[:pout, :pin], src, ident[:pin, :pin])
                        nc.vector.tensor_copy(out=dst, in_=tp[:pout, :pin])
                    P = mupool.tile([128, 2, S], F32, tag="P", name=f"P_{b}")
                    nc.vector.tensor_add(P[:, 0, :], mu_t[:, 0, :], muT[:, 0, :])
                    nc.vector.tensor_add(P[:S1, 1, :], mu_t[:S1, 1, :], muT[:S1, 1, :])
                    muo = mupool.tile([128, 2, S], F32, tag="muo", name=f"muo_{b}")
                    for mc, (m0, mw) in enumerate(((0, S0), (S0, S1))):
                        pp = ps_sm.tile([128, S], F32, tag="sm", name=f"pp_{b}_{mc}")
                        nc.tensor.matmul(pp[:mw, :], P[:, 0, m0:m0 + mw], P[:, 0, :],
                                         start=True, stop=False)
                        nc.tensor.matmul(pp[:mw, :], P[:S1, 1, m0:m0 + mw], P[:S1, 1, :],
                                         start=False, stop=True)
                        nc.scalar.activation(out=muo[:mw, mc, :], in_=pp[:mw, :],
                                             func=AF.Identity, scale=0.25)
                        nc.sync.dma_start(out=o_mu[b, m0:m0 + mw, :], in_=muo[:mw, mc, :])

                # aff chunks with ego_branch materialization
                for Mc in range(OCH):
                    M = ECH + Mc
                    pt = ps_gemm.tile([128, 392], F32, tag="pt", name=f"pt_e{pair}_{M}")
                    for K in range(KC):
                        nc.tensor.matmul(
                            pt[:], w_ego[:, K, M * 128:(M + 1) * 128], xp[:, K, :, :],
                            start=(K == 0), stop=(K == KC - 1),
                        )
                    for i in range(2):
                        b = b0 + i
                        nc.vector.tensor_mul(ebr[:, Mc, b, :],
                                             pt[:, i * 196:(i + 1) * 196], sc_bc[:, i, :])
                        nc.vector.reduce_sum(out=z_ego[:, Mc, b:b + 1],
                                             in_=ebr[:, Mc, b, :], axis=AX.X)
                        nc.scalar.activation(out=ebr[:, Mc, b, :], in_=ebr[:, Mc, b, :],
                                             func=AF.Identity, bias=baffe_sb[:, Mc:Mc + 1])
                        nc.sync.dma_start(out=o_ebr[b, Mc * 128:(Mc + 1) * 128, :],
                                          in_=ebr[:, Mc, b, :])

                # cam rows for this pair
                for i in range(2):
                    b = b0 + i
                    cps = ps_sm.tile([1, S], F32, tag="sm", name=f"cam_{b}")
                    for Kc in range(OCH):
                        nc.tensor.matmul(cps[:], fcwgT_sb[:, Kc, b:b + 1], ebr[:, Kc, b, :],
                                         start=(Kc == 0), stop=(Kc == OCH - 1))
                    cam_b = rowpool.tile([1, S], F32, tag="rscr", name=f"camr_{b}")
                    nc.scalar.activation(out=cam_b, in_=cps, func=AF.Identity,
                                         scale=1.0 / 512.0)
                    nc.sync.dma_start(out=o_cam[b:b + 1, :], in_=cam_b[:])

            # ---- ego epilogue ----
            for Kc in range(OCH):
                nc.vector.tensor_scalar(
                    out=m0_t[:, Kc, :], in0=z_ego[:, Kc, :],
                    scalar1=1.0 / 196.0, scalar2=baffe_sb[:, Kc:Kc + 1],
                    op0=ALU.mult, op1=ALU.add,
                )
            es_ps = ps_sm.tile([EBL, NCL], F32, tag="sm", name="es_ps")
            for Kc in range(OCH):
                nc.tensor.matmul(es_ps[:], m0_t[:, Kc, :], fcwT_sb[:, Kc, :],
                                 start=(Kc == 0), stop=(Kc == OCH - 1))
            fcb_bc = persist.tile([EBL, NCL], F32)
            nc.gpsimd.partition_broadcast(fcb_bc, fcb_sb, channels=EBL)
            es_sb = persist.tile([EBL, NCL], F32)
            nc.vector.tensor_add(es_sb, es_ps, fcb_bc)
            nc.sync.dma_start(out=o_escore[:], in_=es_sb[:])

            logits(img_ego, EBL, ls4, o_lpi, "ego")

            # ---- sim loss ----
            a_t = persist.tile([12, S], F32)
            for v in range(3):
                nc.sync.dma_start(out=a_t[v * 4:(v + 1) * 4, :], in_=attego[:])
            ssa = persist.tile([4, 1], F32)
            scr4 = persist.tile([4, S], F32, name="scr4_sim")
            nc.vector.tensor_mul(scr4, attego, attego)
            nc.vector.reduce_sum(out=ssa, in_=scr4, axis=AX.X)
            nc.scalar.activation(out=ssa, in_=ssa, func=AF.Sqrt)
            na12 = persist.tile([12, 1], F32)
            for v in range(3):
                nc.sync.dma_start(out=na12[v * 4:(v + 1) * 4, :], in_=ssa[:])
            ssb = persist.tile([12, 1], F32)
            scr12 = persist.tile([12, S], F32, name="scr12_sim")
            nc.vector.tensor_mul(scr12, attexo, attexo)
            nc.vector.reduce_sum(out=ssb, in_=scr12, axis=AX.X)
            nc.scalar.activation(out=ssb, in_=ssb, func=AF.Sqrt)
            dt = persist.tile([12, 1], F32)
            nc.vector.tensor_mul(scr12, a_t, attexo)
            nc.vector.reduce_sum(out=dt, in_=scr12, axis=AX.X)
            prod = persist.tile([12, 1], F32)
            nc.vector.tensor_mul(prod, na12, ssb)
            nc.vector.tensor_scalar_max(prod, prod, 1e-8)
            nc.vector.reciprocal(out=prod, in_=prod)
            cosv = persist.tile([12, 1], F32)
            nc.vector.tensor_mul(cosv, dt, prod)
            sim_t = persist.tile([12, 1], F32)
            nc.scalar.activation(out=sim_t, in_=cosv, func=AF.Relu, scale=-1.0, bias=1.0)
            nc.sync.dma_start(out=o_sim[:], in_=sim_t[:])

    nc.compile()
    return nc


_NC_CACHE = {}


def _get_nc():
    if "nc" not in _NC_CACHE:
        _NC_CACHE["nc"] = build_nc()
    return _NC_CACHE["nc"]


def kernel(exo_proj, ego_proj, text_features, W_pool_exo, W_pool_ego,
           W_aff_exo, b_aff_exo, W_aff_ego, b_aff_ego,
           fc_w, fc_b, fc_exo_w, fc_exo_b,
           logit_scale_ego, logit_scale_exo, label):
    nc = _get_nc()
    f32 = np.float32
    exo_proj = np.asarray(exo_proj, f32).reshape(NV, EB, C, S)
    ego_proj = np.asarray(ego_proj, f32).reshape(EB, C, S)
    text_features = np.asarray(text_features, f32)
    label_np = np.asarray(label).astype(np.int64)

    w_exo_cat = np.concatenate(
        [np.asarray(W_pool_exo, f32), np.asarray(W_aff_exo, f32).T], axis=1
    ).astype(ml_dtypes.bfloat16)
    w_ego_cat = np.concatenate(
        [np.asarray(W_pool_ego, f32), np.asarray(W_aff_ego, f32).T], axis=1
    ).astype(ml_dtypes.bfloat16)
    txtT = np.ascontiguousarray(text_features.T)
    fcwT = np.ascontiguousarray(np.asarray(fc_w, f32).T)
    fcxT = np.ascontiguousarray(np.asarray(fc_exo_w, f32).T)
    fcb_row = np.asarray(fc_b, f32).reshape(1, NCL)
    fcxb_row = np.asarray(fc_exo_b, f32).reshape(1, NCL)
    baffx = np.ascontiguousarray(np.asarray(b_aff_exo, f32).reshape(OCH, 128).T)
    baffe = np.ascontiguousarray(np.asarray(b_aff_ego, f32).reshape(OCH, 128).T)
    ls_row = np.array([[np.float32(logit_scale_ego), np.float32(logit_scale_exo)]], f32)

    in_maps = []
    for k in range(NCORES):
        sl = slice(k * EBL, (k + 1) * EBL)
        lab = label_np[sl]
        xexo = np.ascontiguousarray(
            exo_proj[:, sl].reshape(BXL, C, S)).astype(ml_dtypes.bfloat16)
        xego = np.ascontiguousarray(ego_proj[sl]).astype(ml_dtypes.bfloat16)
        tgT = np.ascontiguousarray(text_features[lab].T).astype(ml_dtypes.bfloat16)
        fcwgT = np.ascontiguousarray(np.asarray(fc_w, f32)[lab].T)
        in_maps.append({
            "xexo": xexo, "xego": xego,
            "wexo": w_exo_cat, "wego": w_ego_cat,
            "txt": text_features, "txtT": txtT, "tgT": tgT,
            "fcwgT": fcwgT, "fcwT": fcwT, "fcxT": fcxT,
            "fcb": fcb_row, "fcxb": fcxb_row,
            "baffx": baffx, "baffe": baffe, "ls": ls_row,
        })

    res = run_bass_kernel_spmd(nc, in_maps, core_ids=list(range(NCORES)))
    rs = res.results

    exo_score = np.empty((NV * EB, NCL), f32)
    lpix = np.empty((NV * EB, NCL), f32)
    sim_loss = np.empty((NV * EB,), f32)
    ego_score = np.empty((EB, NCL), f32)
    lpi = np.empty((EB, NCL), f32)
    cam = np.empty((EB, H, W), f32)
    ego_branch = np.empty((EB, O, H, W), f32)
    mu = np.empty((EB, S, S), f32)
    for k in range(NCORES):
        r = rs[k]
        sl = slice(k * EBL, (k + 1) * EBL)
        ego_score[sl] = r["escore"]
        lpi[sl] = r["lpi"]
        cam[sl] = r["cam"].reshape(EBL, H, W)
        ego_branch[sl] = r["ebr"].reshape(EBL, O, H, W)
        mu[sl] = r["mu"]
        # local exo row v*EBL+j  ->  global row v*EB + (k*EBL+j)
        xs = r["xscore"].reshape(NV, EBL, NCL)
        lx = r["lpix"].reshape(NV, EBL, NCL)
        sm = r["sim"].reshape(NV, EBL)
        for v in range(NV):
            gsl = slice(v * EB + k * EBL, v * EB + (k + 1) * EBL)
            exo_score[gsl] = xs[v]
            lpix[gsl] = lx[v]
            sim_loss[gsl] = sm[v]

    logits_per_text = np.ascontiguousarray(lpi.T)
    logits_per_text_exo = np.ascontiguousarray(lpix.T)
    return (exo_score, ego_score, logits_per_text, lpi,
            logits_per_text_exo, lpix, sim_loss, cam, ego_branch, mu)
